# revision 4
# baseline (speedup 1.0000x reference)
"""BitNetV3 MLP kernel for 8 Trainium2 NeuronCores.

Data-parallel over tokens (8 x 512). v2: restructured for engine overlap.

Key changes vs v1 (1701us):
- Phase 0/1 reordered: x-FWHT runs on vector/gpsimd BEFORE the prepass
  reduces in queue order, so the weight-scale AllReduce latency hides
  under phase-1 compute. First matmul ~140us (was 239us).
- Phase 3 (FWHT over I=8192) rewritten: h rows are PE-transposed to
  [I_part, tok] layout first; the FWHT factorizes as
  H_64 (tile axis, 6 butterfly stages split vector/gpsimd) (x)
  H_128 (partition axis, exact f32 PE matmuls vs a +-1 Hadamard
  stationary). Per-token absmax/quant done in the transposed layout via
  strided reduce + PE row-broadcasts. Was 617us of vector-bound wall
  with PE idle; now ~4x less vector work in that stretch.
- Phase 4 consumes activation k-slices directly from the per-token-tile
  transposed bf16 tiles (no separate q2T copy step).

Exactness notes unchanged from v1: int8 casts are rint+sat so they
match jnp.round+clip; ternary weights and int8 activations are exact in
bf16; 1/scales fold into per-token post-scales; FWHT butterflies f32;
the H_128 stage uses the PE fp32 path (2-pass split, ~2^-16 relative)
which perturbs quantization decisions by well under the error budget.
"""

import sys
import numpy as np

if "/opt/trn_rl_repo" not in sys.path:
    sys.path.insert(0, "/opt/trn_rl_repo")

B, S, H, I = 2, 2048, 2048, 8192
N_CORES = 8
T = (B * S) // N_CORES  # tokens per core = 512
TT = T // 128           # token tiles per core = 4
EPS = 1e-5
NORM_H = float(1.0 / np.sqrt(H))
NORM_I = float(1.0 / np.sqrt(I))

KH = H // 128   # 16 k-tiles (gate/up contraction)
KI = I // 128   # 64 k-tiles (down contraction)
OBLK = 512
NOB_GU = I // OBLK   # 16
NOB_D = H // OBLK    # 4
AI = I // 128        # 64 column-blocks (a axis) in transposed layout

_CACHE = {}


def _butterfly(nc, Alu, src, dst, h):
    """One butterfly stage (pairs at stride h), all on vector."""
    ca = src.rearrange("p (n two h) -> p n two h", two=2, h=h)
    na = dst.rearrange("p (n two h) -> p n two h", two=2, h=h)
    a = ca[:, :, 0, :]
    b = ca[:, :, 1, :]
    nc.vector.tensor_tensor(na[:, :, 0, :], a, b, Alu.add)
    nc.vector.tensor_tensor(na[:, :, 1, :], a, b, Alu.subtract)


def _butterfly_vg(nc, Alu, src, dst, h):
    """One butterfly stage split ~half/half across vector and gpsimd."""
    ca = src.rearrange("p (n two h) -> p n two h", two=2, h=h)
    na = dst.rearrange("p (n two h) -> p n two h", two=2, h=h)
    n = ca.shape[1]
    if h >= 2:
        hv = h // 2
        for eng, sl in ((nc.vector, slice(0, hv)), (nc.gpsimd, slice(hv, h))):
            a = ca[:, :, 0, sl]
            b = ca[:, :, 1, sl]
            eng.tensor_tensor(na[:, :, 0, sl], a, b, Alu.add)
            eng.tensor_tensor(na[:, :, 1, sl], a, b, Alu.subtract)
    else:
        nv = n // 2
        for eng, sl in ((nc.vector, slice(0, nv)), (nc.gpsimd, slice(nv, n))):
            a = ca[:, sl, 0, :]
            b = ca[:, sl, 1, :]
            eng.tensor_tensor(na[:, sl, 0, :], a, b, Alu.add)
            eng.tensor_tensor(na[:, sl, 1, :], a, b, Alu.subtract)


def _build_program():
    import concourse.mybir as mybir
    from concourse import bacc
    from concourse.bass import broadcast_tensor_aps
    from concourse.tile import TileContext
    from concourse.masks import make_identity

    f32 = mybir.dt.float32
    bf16 = mybir.dt.bfloat16
    i8 = mybir.dt.int8
    Alu = mybir.AluOpType
    Act = mybir.ActivationFunctionType
    Axis = mybir.AxisListType

    nc = bacc.Bacc("TRN2", target_bir_lowering=False, debug=False,
                   num_devices=N_CORES)

    x = nc.dram_tensor("x", [T, H], f32, kind="ExternalInput").ap()
    wgT = nc.dram_tensor("wgT", [H, I], f32, kind="ExternalInput").ap()
    wuT = nc.dram_tensor("wuT", [H, I], f32, kind="ExternalInput").ap()
    wdT = nc.dram_tensor("wdT", [I, H], f32, kind="ExternalInput").ap()
    h128 = nc.dram_tensor("h128", [128, 128], f32, kind="ExternalInput").ap()
    # per-core 1/8 slices for the global-scale prepass
    wg_pre = nc.dram_tensor("wg_pre", [H, I // 8], f32, kind="ExternalInput").ap()
    wu_pre = nc.dram_tensor("wu_pre", [H, I // 8], f32, kind="ExternalInput").ap()
    wd_pre = nc.dram_tensor("wd_pre", [I, H // 8], f32, kind="ExternalInput").ap()
    out = nc.dram_tensor("out", [T, H], f32, kind="ExternalOutput").ap()

    h2d = nc.dram_tensor("h2d", [T, I], f32).ap()  # spilled h = silu(g)*u
    cc_in = nc.dram_tensor("cc_in", [1, 8], f32)
    cc_out = nc.dram_tensor("cc_out", [1, 8], f32, addr_space="Shared")

    with TileContext(nc) as tc:
        with (
            tc.tile_pool(name="const", bufs=1) as cpool,
            tc.tile_pool(name="scal", bufs=1) as spool,
            tc.tile_pool(name="psum", bufs=6, space="PSUM") as ppool,
            tc.tile_pool(name="psum_tr", bufs=2, space="PSUM") as trpool,
        ):
            # ---------------- constants ----------------
            ident_bf = cpool.tile([128, 128], bf16)
            make_identity(nc, ident_bf[:])
            ident_f = cpool.tile([128, 128], f32)
            make_identity(nc, ident_f[:])
            ones_row = cpool.tile([1, 128], f32)
            nc.vector.memset(ones_row[:], 1.0)
            h128_sb = cpool.tile([128, 128], f32)
            nc.sync.dma_start(out=h128_sb[:], in_=h128[:, :])

            comb_g, comb_u, comb_d = [], [], []
            ipost1 = []  # per-tt [128,1] act post-scale (phase 1)

            # ---------- phase 1: x -> fwht -> quant -> q1T ----------
            # (emitted BEFORE the prepass so vector work starts on x
            # immediately; prepass reduces queue behind it while its
            # DMA streams, and the AllReduce hides under this.)
            with tc.tile_pool(name="q1T", bufs=1) as q1Tpool:
                q1T = [q1Tpool.tile([128, T], bf16, tag=f"q1T_{k}",
                                    name=f"q1T_{k}") for k in range(KH)]
                with tc.tile_pool(name="xb", bufs=2) as xpool, \
                     tc.tile_pool(name="q1s", bufs=2) as q1pool:
                    for tt in range(TT):
                        xa = xpool.tile([128, H], f32, tag="xa")
                        xb2 = xpool.tile([128, H], f32, tag="xb2")
                        nc.sync.dma_start(out=xa[:],
                                          in_=x[128 * tt:128 * (tt + 1)])
                        cur, nxt = xa[:], xb2[:]
                        h = 1
                        while h < H:
                            _butterfly_vg(nc, Alu, cur, nxt, h)
                            cur, nxt = nxt, cur
                            h *= 2
                        amax = spool.tile([128, 1], f32, tag=f"amax1_{tt}")
                        nc.vector.tensor_reduce(amax[:], cur, Axis.X, Alu.max,
                                                apply_absolute_value=True)
                        a_c = spool.tile([128, 1], f32, tag=f"ac1_{tt}")
                        nc.vector.tensor_scalar(a_c[:], amax[:], NORM_H, EPS,
                                                Alu.mult, Alu.max)
                        ipost = spool.tile([128, 1], f32, tag=f"ip1_{tt}")
                        nc.vector.tensor_scalar_mul(ipost[:], a_c[:],
                                                    1.0 / 127.0)
                        ipost1.append(ipost)
                        r1 = spool.tile([128, 1], f32, tag=f"r1_{tt}")
                        nc.vector.reciprocal(r1[:], ipost[:])
                        qs = spool.tile([128, 1], f32, tag=f"qs1_{tt}")
                        nc.vector.tensor_scalar_mul(qs[:], r1[:], NORM_H)
                        q_i8 = q1pool.tile([128, H], i8, tag="q1i8")
                        nc.scalar.activation(q_i8[:], cur, Act.Copy,
                                             scale=qs[:])
                        q_bf = q1pool.tile([128, H], bf16, tag="q1bf")
                        nc.vector.tensor_copy(q_bf[:], q_i8[:])
                        for g in range(KH // 4):
                            ps = trpool.tile([128, 512], bf16, tag="tr")
                            for s4 in range(4):
                                k = 4 * g + s4
                                nc.tensor.transpose(
                                    ps[:, 128 * s4:128 * (s4 + 1)],
                                    q_bf[:, 128 * k:128 * (k + 1)],
                                    ident_bf[:])
                            for s4 in range(4):
                                k = 4 * g + s4
                                nc.vector.tensor_copy(
                                    q1T[k][:, 128 * tt:128 * (tt + 1)],
                                    ps[:, 128 * s4:128 * (s4 + 1)])

                # ---------------- phase 0: global weight scales -----------
                st = spool.tile([1, 8], f32)
                nc.vector.memset(st[:], 0.0)
                with tc.tile_pool(name="pre", bufs=4) as prepool:
                    def abs_sum_slice(wpre, idx):
                        rows, cols = wpre.shape
                        ntile = rows // 128
                        acc = spool.tile([128, 1], f32, tag=f"acc{idx}")
                        for k in range(ntile):
                            wtile = prepool.tile([128, cols], f32, tag="pre")
                            nc.sync.dma_start(out=wtile[:],
                                              in_=wpre[128 * k:128 * (k + 1)])
                            part = spool.tile([128, 1], f32, tag=f"part{idx}")
                            nc.vector.tensor_reduce(part[:], wtile[:], Axis.X,
                                                    Alu.add,
                                                    apply_absolute_value=True)
                            if k == 0:
                                nc.vector.tensor_copy(acc[:], part[:])
                            else:
                                nc.vector.tensor_tensor(acc[:], acc[:],
                                                        part[:], Alu.add)
                        tot = spool.tile([1, 1], f32, tag=f"tot{idx}")
                        nc.gpsimd.tensor_reduce(tot[:], acc[:], Axis.C,
                                                Alu.add)
                        nc.vector.tensor_copy(st[0:1, idx:idx + 1], tot[:])

                    abs_sum_slice(wg_pre, 0)
                    abs_sum_slice(wu_pre, 1)
                    abs_sum_slice(wd_pre, 2)

                nc.sync.dma_start(out=cc_in[:], in_=st[:])
                nc.gpsimd.collective_compute(
                    "AllReduce", Alu.add, ins=[cc_in[:]], outs=[cc_out[:]],
                    replica_groups=[list(range(N_CORES))])
                sums = spool.tile([1, 8], f32)
                nc.sync.dma_start(out=sums[:], in_=cc_out[:])

                # s_w = max(mean, EPS); inv_s = 1/s_w
                means = spool.tile([1, 8], f32)
                nc.vector.tensor_scalar(means[:], sums[:], 1.0 / (H * I), EPS,
                                        Alu.mult, Alu.max)
                invs = spool.tile([1, 8], f32)
                nc.vector.reciprocal(invs[:], means[:])
                bc_ps = trpool.tile([128, 8], f32, tag="tr")
                nc.tensor.matmul(bc_ps[:], ones_row[:], means[:],
                                 start=True, stop=True)
                s_w_bc = spool.tile([128, 8], f32)
                nc.vector.tensor_copy(s_w_bc[:], bc_ps[:])
                bc_ps2 = trpool.tile([128, 8], f32, tag="tr")
                nc.tensor.matmul(bc_ps2[:], ones_row[:], invs[:],
                                 start=True, stop=True)
                inv_w_bc = spool.tile([128, 8], f32)
                nc.vector.tensor_copy(inv_w_bc[:], bc_ps2[:])

                # combined per-token post-scales for gate/up
                for tt in range(TT):
                    cg = spool.tile([128, 1], f32, tag=f"cg_{tt}")
                    nc.vector.tensor_tensor(cg[:], ipost1[tt][:],
                                            s_w_bc[:, 0:1], Alu.mult)
                    comb_g.append(cg)
                    cu = spool.tile([128, 1], f32, tag=f"cu_{tt}")
                    nc.vector.tensor_tensor(cu[:], ipost1[tt][:],
                                            s_w_bc[:, 1:2], Alu.mult)
                    comb_u.append(cu)

                # ---------- phase 2: gate/up matmuls + silu*up -> DRAM ----
                with tc.tile_pool(name="wload", bufs=8) as wpool, \
                     tc.tile_pool(name="tern", bufs=8) as tpool, \
                     tc.tile_pool(name="gsb", bufs=8) as gpool, \
                     tc.tile_pool(name="hst", bufs=8) as hpool:
                    for ob in range(NOB_GU):
                        osl = slice(OBLK * ob, OBLK * (ob + 1))
                        ps_g = [ppool.tile([128, OBLK], f32, tag="mm",
                                           name="ps_g") for _ in range(TT)]
                        for k in range(KH):
                            wti = wpool.tile([128, OBLK], f32, tag="w")
                            nc.sync.dma_start(
                                out=wti[:],
                                in_=wgT[128 * k:128 * (k + 1), osl])
                            t_i8 = tpool.tile([128, OBLK], i8, tag="ti8")
                            nc.scalar.activation(t_i8[:], wti[:], Act.Copy,
                                                 scale=inv_w_bc[:, 0:1])
                            t_bf = tpool.tile([128, OBLK], bf16, tag="tbf")
                            nc.vector.tensor_scalar(t_bf[:], t_i8[:], -1.0,
                                                    1.0, Alu.max, Alu.min)
                            for tt in range(TT):
                                nc.tensor.matmul(
                                    ps_g[tt][:],
                                    q1T[k][:, 128 * tt:128 * (tt + 1)],
                                    t_bf[:], start=(k == 0),
                                    stop=(k == KH - 1))
                        gate_sb = []
                        for tt in range(TT):
                            g = gpool.tile([128, OBLK], f32, tag="gate")
                            nc.scalar.activation(g[:], ps_g[tt][:], Act.Silu,
                                                 scale=comb_g[tt][:])
                            gate_sb.append(g)
                        ps_u = [ppool.tile([128, OBLK], f32, tag="mm",
                                           name="ps_u") for _ in range(TT)]
                        for k in range(KH):
                            wti = wpool.tile([128, OBLK], f32, tag="w")
                            nc.sync.dma_start(
                                out=wti[:],
                                in_=wuT[128 * k:128 * (k + 1), osl])
                            t_i8 = tpool.tile([128, OBLK], i8, tag="ti8")
                            nc.scalar.activation(t_i8[:], wti[:], Act.Copy,
                                                 scale=inv_w_bc[:, 1:2])
                            t_bf = tpool.tile([128, OBLK], bf16, tag="tbf")
                            nc.vector.tensor_scalar(t_bf[:], t_i8[:], -1.0,
                                                    1.0, Alu.max, Alu.min)
                            for tt in range(TT):
                                nc.tensor.matmul(
                                    ps_u[tt][:],
                                    q1T[k][:, 128 * tt:128 * (tt + 1)],
                                    t_bf[:], start=(k == 0),
                                    stop=(k == KH - 1))
                        for tt in range(TT):
                            hs = hpool.tile([128, OBLK], f32, tag="hst")
                            nc.vector.scalar_tensor_tensor(
                                hs[:], ps_u[tt][:], comb_u[tt][:],
                                gate_sb[tt][:], Alu.mult, Alu.mult)
                            nc.sync.dma_start(
                                out=h2d[128 * tt:128 * (tt + 1), osl],
                                in_=hs[:])

            # ---------- phase 3: h -> [I,tok] fwht -> quant (transposed) --
            with tc.tile_pool(name="q2big", bufs=1) as q2bpool:
                Qq = [q2bpool.tile([128, I], bf16, tag=f"Qq{t}",
                                   name=f"Qq{t}") for t in range(TT)]
                with tc.tile_pool(name="hrow", bufs=1) as rpool, \
                     tc.tile_pool(name="ph3", bufs=1) as p3pool:
                    for tt in range(TT):
                        hrow = rpool.tile([128, I], f32, tag="hrow")
                        nc.sync.dma_start(out=hrow[:],
                                          in_=h2d[128 * tt:128 * (tt + 1)])
                        Q = p3pool.tile([128, I], f32, tag="Q")
                        Q2 = p3pool.tile([128, I], f32, tag="Q2")
                        # transpose 64 blocks: Q[b, 128a+t] = h[t, 128a+b]
                        for g in range(AI // 4):
                            ps = trpool.tile([128, 512], f32, tag="tr")
                            for s4 in range(4):
                                a = 4 * g + s4
                                nc.tensor.transpose(
                                    ps[:, 128 * s4:128 * (s4 + 1)],
                                    hrow[:, 128 * a:128 * (a + 1)],
                                    ident_f[:])
                            nc.scalar.activation(
                                Q[:, 512 * g:512 * (g + 1)], ps[:], Act.Copy)
                        # 6 cross-tile butterfly stages over a (v/g split)
                        cur, nxt = Q[:], Q2[:]
                        for s in (1, 2, 4, 8, 16, 32):
                            _butterfly_vg(nc, Alu, cur, nxt, 128 * s)
                            cur, nxt = nxt, cur
                        # cur is Q again (6 stages). H_128 over partitions
                        # via f32 PE matmuls; evacuate psum into Q2.
                        for c in range(I // 512):
                            psH = ppool.tile([128, 512], f32, tag="mm",
                                             name="psH")
                            nc.tensor.matmul(psH[:], h128_sb[:],
                                             cur[:, 512 * c:512 * (c + 1)],
                                             start=True, stop=True)
                            nc.scalar.activation(
                                Q2[:, 512 * c:512 * (c + 1)], psH[:],
                                Act.Copy)
                        # per-token absmax: reduce over a (strided), then
                        # over partitions.
                        q3 = Q2.rearrange("p (a t) -> p t a", a=AI)
                        tmax = spool.tile([128, 128], f32, tag="tmax")
                        nc.vector.tensor_reduce(tmax[:], q3, Axis.X, Alu.max,
                                                apply_absolute_value=True)
                        arow = spool.tile([1, 128], f32, tag=f"arow_{tt}")
                        nc.gpsimd.tensor_reduce(arow[:], tmax[:], Axis.C,
                                                Alu.max)
                        a_c = spool.tile([1, 128], f32, tag=f"ac2_{tt}")
                        nc.vector.tensor_scalar(a_c[:], arow[:], NORM_I, EPS,
                                                Alu.mult, Alu.max)
                        ipost = spool.tile([1, 128], f32, tag=f"ip2_{tt}")
                        nc.vector.tensor_scalar_mul(ipost[:], a_c[:],
                                                    1.0 / 127.0)
                        r2 = spool.tile([1, 128], f32, tag=f"r2_{tt}")
                        nc.vector.reciprocal(r2[:], ipost[:])
                        qs_row = spool.tile([1, 128], f32, tag=f"qs2_{tt}")
                        nc.vector.tensor_scalar_mul(qs_row[:], r2[:], NORM_I)
                        comb_row = spool.tile([1, 128], f32,
                                              tag=f"cdr_{tt}")
                        nc.scalar.activation(comb_row[:], ipost[:], Act.Copy,
                                             scale=s_w_bc[0:1, 2:3])
                        # broadcast qs to [128,128]; comb to [128,1]
                        psB = trpool.tile([128, 128], f32, tag="tr")
                        nc.tensor.matmul(psB[:], ones_row[:], qs_row[:],
                                         start=True, stop=True)
                        qs_bc = spool.tile([128, 128], f32, tag="qsbc")
                        nc.vector.tensor_copy(qs_bc[:], psB[:])
                        psC = trpool.tile([128, 1], f32, tag="tr")
                        nc.tensor.matmul(psC[:], comb_row[:],
                                         ones_row[:, 0:1],
                                         start=True, stop=True)
                        cd = spool.tile([128, 1], f32, tag=f"cd_{tt}")
                        nc.vector.tensor_copy(cd[:], psC[:])
                        comb_d.append(cd)
                        # quantize: Q = Q2 * qs (broadcast over a), then
                        # rint+sat to i8 (scalar), then bf16 (vector).
                        q3o = Q.rearrange("p (a t) -> p a t", a=AI)
                        q3i = Q2.rearrange("p (a t) -> p a t", a=AI)
                        s3 = qs_bc.rearrange("p (o t) -> p o t", o=1)
                        b_in, b_s = broadcast_tensor_aps(q3i, s3)
                        nc.vector.tensor_tensor(q3o, b_in, b_s, Alu.mult)
                        qi8 = p3pool.tile([128, I], i8, tag="qi8")
                        nc.scalar.activation(qi8[:], Q[:], Act.Copy)
                        nc.vector.tensor_copy(Qq[tt][:], qi8[:])

                # ---------- phase 4: down matmul ----------
                with tc.tile_pool(name="wload4", bufs=8) as wpool, \
                     tc.tile_pool(name="tern4", bufs=8) as tpool, \
                     tc.tile_pool(name="osb", bufs=8) as opool:
                    for ob in range(NOB_D):
                        osl = slice(OBLK * ob, OBLK * (ob + 1))
                        ps_d = [ppool.tile([128, OBLK], f32, tag="mm",
                                           name="ps_d") for _ in range(TT)]
                        for k in range(KI):
                            wti = wpool.tile([128, OBLK], f32, tag="w")
                            nc.sync.dma_start(
                                out=wti[:],
                                in_=wdT[128 * k:128 * (k + 1), osl])
                            t_i8 = tpool.tile([128, OBLK], i8, tag="ti8")
                            nc.scalar.activation(t_i8[:], wti[:], Act.Copy,
                                                 scale=inv_w_bc[:, 2:3])
                            t_bf = tpool.tile([128, OBLK], bf16, tag="tbf")
                            nc.vector.tensor_scalar(t_bf[:], t_i8[:], -1.0,
                                                    1.0, Alu.max, Alu.min)
                            for tt in range(TT):
                                nc.tensor.matmul(
                                    ps_d[tt][:],
                                    Qq[tt][:, 128 * k:128 * (k + 1)],
                                    t_bf[:], start=(k == 0),
                                    stop=(k == KI - 1))
                        for tt in range(TT):
                            o_sb = opool.tile([128, OBLK], f32, tag="out")
                            nc.scalar.activation(o_sb[:], ps_d[tt][:],
                                                 Act.Copy,
                                                 scale=comb_d[tt][:])
                            nc.sync.dma_start(
                                out=out[128 * tt:128 * (tt + 1), osl],
                                in_=o_sb[:])

    nc.compile()
    return nc


def _get_program():
    if "nc" not in _CACHE:
        _CACHE["nc"] = _build_program()
    return _CACHE["nc"]


def _hadamard128():
    idx = np.arange(128, dtype=np.int64)
    anded = idx[:, None] & idx[None, :]
    pc = np.zeros_like(anded)
    for b in range(7):
        pc += (anded >> b) & 1
    return (1.0 - 2.0 * (pc % 2)).astype(np.float32)


def _make_in_maps(hidden_states, w_gate, w_up, w_down):
    x2 = np.ascontiguousarray(hidden_states.reshape(B * S, H),
                              dtype=np.float32)
    wgT = np.ascontiguousarray(np.asarray(w_gate, dtype=np.float32).T)
    wuT = np.ascontiguousarray(np.asarray(w_up, dtype=np.float32).T)
    wdT = np.ascontiguousarray(np.asarray(w_down, dtype=np.float32).T)
    h128 = _hadamard128()

    ci = I // 8
    ch = H // 8
    in_maps = [
        {
            "x": np.ascontiguousarray(x2[T * j:T * (j + 1)]),
            "wgT": wgT, "wuT": wuT, "wdT": wdT, "h128": h128,
            "wg_pre": np.ascontiguousarray(wgT[:, ci * j:ci * (j + 1)]),
            "wu_pre": np.ascontiguousarray(wuT[:, ci * j:ci * (j + 1)]),
            "wd_pre": np.ascontiguousarray(wdT[:, ch * j:ch * (j + 1)]),
        }
        for j in range(N_CORES)
    ]
    return in_maps


def kernel(hidden_states, w_gate, w_up, w_down, _trace=False):
    from concourse.bass_utils import run_bass_kernel_spmd

    nc = _get_program()
    in_maps = _make_in_maps(hidden_states, w_gate, w_up, w_down)
    res = run_bass_kernel_spmd(nc, in_maps, list(range(N_CORES)),
                               trace=_trace)
    pieces = [res.results[j]["out"] for j in range(N_CORES)]
    out = np.concatenate(pieces, axis=0).reshape(B, S, H)
    out = np.ascontiguousarray(out, dtype=np.float32)
    if _trace:
        return out, res
    return out


# revision 7
# speedup vs baseline: 1.1059x; 1.1059x over previous
"""BitNetV3 MLP kernel for 8 Trainium2 NeuronCores.

Data-parallel over tokens (8 x 512). v2: restructured for engine overlap.

Key changes vs v1 (1701us):
- Phase 0/1 reordered: x-FWHT runs on vector/gpsimd BEFORE the prepass
  reduces in queue order, so the weight-scale AllReduce latency hides
  under phase-1 compute. First matmul ~140us (was 239us).
- Phase 3 (FWHT over I=8192) rewritten: h rows are PE-transposed to
  [I_part, tok] layout first; the FWHT factorizes as
  H_64 (tile axis, 6 butterfly stages split vector/gpsimd) (x)
  H_128 (partition axis, exact f32 PE matmuls vs a +-1 Hadamard
  stationary). Per-token absmax/quant done in the transposed layout via
  strided reduce + PE row-broadcasts. Was 617us of vector-bound wall
  with PE idle; now ~4x less vector work in that stretch.
- Phase 4 consumes activation k-slices directly from the per-token-tile
  transposed bf16 tiles (no separate q2T copy step).

Exactness notes unchanged from v1: int8 casts are rint+sat so they
match jnp.round+clip; ternary weights and int8 activations are exact in
bf16; 1/scales fold into per-token post-scales; FWHT butterflies f32;
the H_128 stage uses the PE fp32 path (2-pass split, ~2^-16 relative)
which perturbs quantization decisions by well under the error budget.
"""

import sys
import numpy as np

if "/opt/trn_rl_repo" not in sys.path:
    sys.path.insert(0, "/opt/trn_rl_repo")

B, S, H, I = 2, 2048, 2048, 8192
N_CORES = 8
T = (B * S) // N_CORES  # tokens per core = 512
TT = T // 128           # token tiles per core = 4
EPS = 1e-5
NORM_H = float(1.0 / np.sqrt(H))
NORM_I = float(1.0 / np.sqrt(I))

KH = H // 128   # 16 k-tiles (gate/up contraction)
KI = I // 128   # 64 k-tiles (down contraction)
OBLK = 512
NOB_GU = I // OBLK   # 16
NOB_D = H // OBLK    # 4
AI = I // 128        # 64 column-blocks (a axis) in transposed layout

_CACHE = {}


def _butterfly(nc, Alu, src, dst, h):
    """One butterfly stage (pairs at stride h), all on vector."""
    ca = src.rearrange("p (n two h) -> p n two h", two=2, h=h)
    na = dst.rearrange("p (n two h) -> p n two h", two=2, h=h)
    a = ca[:, :, 0, :]
    b = ca[:, :, 1, :]
    nc.vector.tensor_tensor(na[:, :, 0, :], a, b, Alu.add)
    nc.vector.tensor_tensor(na[:, :, 1, :], a, b, Alu.subtract)


def _butterfly_vg(nc, Alu, src, dst, h):
    """One butterfly stage split ~3:1 across vector and gpsimd (gpsimd
    f32 elementwise is ~3.4x slower than vector, so it gets 1/4)."""
    ca = src.rearrange("p (n two h) -> p n two h", two=2, h=h)
    na = dst.rearrange("p (n two h) -> p n two h", two=2, h=h)
    hv = (3 * h) // 4
    for eng, sl in ((nc.vector, slice(0, hv)), (nc.gpsimd, slice(hv, h))):
        a = ca[:, :, 0, sl]
        b = ca[:, :, 1, sl]
        eng.tensor_tensor(na[:, :, 0, sl], a, b, Alu.add)
        eng.tensor_tensor(na[:, :, 1, sl], a, b, Alu.subtract)


def _build_program():
    import concourse.mybir as mybir
    from concourse import bacc
    from concourse.bass import broadcast_tensor_aps
    from concourse.tile import TileContext
    from concourse.masks import make_identity

    f32 = mybir.dt.float32
    bf16 = mybir.dt.bfloat16
    i8 = mybir.dt.int8
    Alu = mybir.AluOpType
    Act = mybir.ActivationFunctionType
    Axis = mybir.AxisListType

    nc = bacc.Bacc("TRN2", target_bir_lowering=False, debug=False,
                   num_devices=N_CORES)

    x = nc.dram_tensor("x", [T, H], f32, kind="ExternalInput").ap()
    wgT = nc.dram_tensor("wgT", [H, I], f32, kind="ExternalInput").ap()
    wuT = nc.dram_tensor("wuT", [H, I], f32, kind="ExternalInput").ap()
    wdT = nc.dram_tensor("wdT", [I, H], f32, kind="ExternalInput").ap()
    h128 = nc.dram_tensor("h128", [128, 128], f32, kind="ExternalInput").ap()
    # per-core 1/8 slices for the global-scale prepass
    wg_pre = nc.dram_tensor("wg_pre", [H, I // 8], f32, kind="ExternalInput").ap()
    wu_pre = nc.dram_tensor("wu_pre", [H, I // 8], f32, kind="ExternalInput").ap()
    wd_pre = nc.dram_tensor("wd_pre", [I, H // 8], f32, kind="ExternalInput").ap()
    out = nc.dram_tensor("out", [T, H], f32, kind="ExternalOutput").ap()

    h2d = nc.dram_tensor("h2d", [T, I], f32).ap()  # spilled h = silu(g)*u
    cc_in = nc.dram_tensor("cc_in", [1, 8], f32)
    cc_out = nc.dram_tensor("cc_out", [1, 8], f32, addr_space="Shared")

    with TileContext(nc) as tc:
        with (
            tc.tile_pool(name="const", bufs=1) as cpool,
            tc.tile_pool(name="scal", bufs=1) as spool,
            tc.tile_pool(name="psum", bufs=6, space="PSUM") as ppool,
            tc.tile_pool(name="psum_tr", bufs=2, space="PSUM") as trpool,
        ):
            # ---------------- constants ----------------
            ident_bf = cpool.tile([128, 128], bf16)
            make_identity(nc, ident_bf[:])
            ident_f = cpool.tile([128, 128], f32)
            make_identity(nc, ident_f[:])
            ones_row = cpool.tile([1, 128], f32)
            nc.vector.memset(ones_row[:], 1.0)
            h128_sb = cpool.tile([128, 128], f32)
            nc.sync.dma_start(out=h128_sb[:], in_=h128[:, :])

            comb_g, comb_u, comb_d = [], [], []

            # ---------------- phase 0: global weight scales ---------------
            st = spool.tile([1, 8], f32)
            nc.vector.memset(st[:], 0.0)
            with tc.tile_pool(name="pre", bufs=4) as prepool:
                def abs_sum_slice(wpre, idx):
                    rows, cols = wpre.shape
                    ntile = rows // 128
                    acc = spool.tile([128, 1], f32, tag=f"acc{idx}")
                    for k in range(ntile):
                        wtile = prepool.tile([128, cols], f32, tag="pre")
                        nc.sync.dma_start(out=wtile[:],
                                          in_=wpre[128 * k:128 * (k + 1)])
                        part = spool.tile([128, 1], f32, tag=f"part{idx}")
                        nc.vector.tensor_reduce(part[:], wtile[:], Axis.X,
                                                Alu.add,
                                                apply_absolute_value=True)
                        if k == 0:
                            nc.vector.tensor_copy(acc[:], part[:])
                        else:
                            nc.vector.tensor_tensor(acc[:], acc[:], part[:],
                                                    Alu.add)
                    tot = spool.tile([1, 1], f32, tag=f"tot{idx}")
                    nc.gpsimd.tensor_reduce(tot[:], acc[:], Axis.C, Alu.add)
                    nc.vector.tensor_copy(st[0:1, idx:idx + 1], tot[:])

                abs_sum_slice(wg_pre, 0)
                abs_sum_slice(wu_pre, 1)
                abs_sum_slice(wd_pre, 2)

            nc.sync.dma_start(out=cc_in[:], in_=st[:])
            nc.gpsimd.collective_compute(
                "AllReduce", Alu.add, ins=[cc_in[:]], outs=[cc_out[:]],
                replica_groups=[list(range(N_CORES))])
            sums = spool.tile([1, 8], f32)
            nc.sync.dma_start(out=sums[:], in_=cc_out[:])

            # s_w = max(mean, EPS); inv_s = 1/s_w
            means = spool.tile([1, 8], f32)
            nc.vector.tensor_scalar(means[:], sums[:], 1.0 / (H * I), EPS,
                                    Alu.mult, Alu.max)
            invs = spool.tile([1, 8], f32)
            nc.vector.reciprocal(invs[:], means[:])
            bc_ps = trpool.tile([128, 8], f32, tag="tr")
            nc.tensor.matmul(bc_ps[:], ones_row[:], means[:],
                             start=True, stop=True)
            s_w_bc = spool.tile([128, 8], f32)
            nc.vector.tensor_copy(s_w_bc[:], bc_ps[:])
            bc_ps2 = trpool.tile([128, 8], f32, tag="tr")
            nc.tensor.matmul(bc_ps2[:], ones_row[:], invs[:],
                             start=True, stop=True)
            inv_w_bc = spool.tile([128, 8], f32)
            nc.vector.tensor_copy(inv_w_bc[:], bc_ps2[:])

            with tc.tile_pool(name="q1T", bufs=1) as q1Tpool:
                q1T = [q1Tpool.tile([128, T], bf16, tag=f"q1T_{k}",
                                    name=f"q1T_{k}") for k in range(KH)]

                # ---------- phase 1: x -> fwht -> quant -> q1T ----------
                with tc.tile_pool(name="xb", bufs=2) as xpool, \
                     tc.tile_pool(name="q1s", bufs=2) as q1pool:
                    for tt in range(TT):
                        xa = xpool.tile([128, H], f32, tag="xa")
                        xb2 = xpool.tile([128, H], f32, tag="xb2")
                        nc.sync.dma_start(out=xa[:],
                                          in_=x[128 * tt:128 * (tt + 1)])
                        cur, nxt = xa[:], xb2[:]
                        h = 1
                        while h < H:
                            _butterfly(nc, Alu, cur, nxt, h)
                            cur, nxt = nxt, cur
                            h *= 2
                        amax = spool.tile([128, 1], f32, tag=f"amax1_{tt}")
                        nc.vector.tensor_reduce(amax[:], cur, Axis.X, Alu.max,
                                                apply_absolute_value=True)
                        a_c = spool.tile([128, 1], f32, tag=f"ac1_{tt}")
                        nc.vector.tensor_scalar(a_c[:], amax[:], NORM_H, EPS,
                                                Alu.mult, Alu.max)
                        ipost = spool.tile([128, 1], f32, tag=f"ip1_{tt}")
                        nc.vector.tensor_scalar_mul(ipost[:], a_c[:],
                                                    1.0 / 127.0)
                        r1 = spool.tile([128, 1], f32, tag=f"r1_{tt}")
                        nc.vector.reciprocal(r1[:], ipost[:])
                        qs = spool.tile([128, 1], f32, tag=f"qs1_{tt}")
                        nc.vector.tensor_scalar_mul(qs[:], r1[:], NORM_H)
                        cg = spool.tile([128, 1], f32, tag=f"cg_{tt}")
                        nc.vector.tensor_tensor(cg[:], ipost[:],
                                                s_w_bc[:, 0:1], Alu.mult)
                        comb_g.append(cg)
                        cu = spool.tile([128, 1], f32, tag=f"cu_{tt}")
                        nc.vector.tensor_tensor(cu[:], ipost[:],
                                                s_w_bc[:, 1:2], Alu.mult)
                        comb_u.append(cu)
                        q_i8 = q1pool.tile([128, H], i8, tag="q1i8")
                        nc.scalar.activation(q_i8[:], cur, Act.Copy,
                                             scale=qs[:])
                        q_bf = q1pool.tile([128, H], bf16, tag="q1bf")
                        nc.vector.tensor_copy(q_bf[:], q_i8[:])
                        for g in range(KH // 4):
                            ps = trpool.tile([128, 512], bf16, tag="tr")
                            for s4 in range(4):
                                k = 4 * g + s4
                                nc.tensor.transpose(
                                    ps[:, 128 * s4:128 * (s4 + 1)],
                                    q_bf[:, 128 * k:128 * (k + 1)],
                                    ident_bf[:])
                            for s4 in range(4):
                                k = 4 * g + s4
                                nc.vector.tensor_copy(
                                    q1T[k][:, 128 * tt:128 * (tt + 1)],
                                    ps[:, 128 * s4:128 * (s4 + 1)])

                # ---------- phase 2: gate/up matmuls + silu*up -> DRAM ----
                with tc.tile_pool(name="wload", bufs=8) as wpool, \
                     tc.tile_pool(name="tern", bufs=8) as tpool, \
                     tc.tile_pool(name="gsb", bufs=8) as gpool, \
                     tc.tile_pool(name="hst", bufs=8) as hpool:
                    for ob in range(NOB_GU):
                        osl = slice(OBLK * ob, OBLK * (ob + 1))
                        ps_g = [ppool.tile([128, OBLK], f32, tag="mm",
                                           name="ps_g") for _ in range(TT)]
                        for k in range(KH):
                            wti = wpool.tile([128, OBLK], f32, tag="w")
                            nc.sync.dma_start(
                                out=wti[:],
                                in_=wgT[128 * k:128 * (k + 1), osl])
                            t_i8 = tpool.tile([128, OBLK], i8, tag="ti8")
                            nc.scalar.activation(t_i8[:], wti[:], Act.Copy,
                                                 scale=inv_w_bc[:, 0:1])
                            t_bf = tpool.tile([128, OBLK], bf16, tag="tbf")
                            nc.vector.tensor_scalar(t_bf[:], t_i8[:], -1.0,
                                                    1.0, Alu.max, Alu.min)
                            for tt in range(TT):
                                nc.tensor.matmul(
                                    ps_g[tt][:],
                                    q1T[k][:, 128 * tt:128 * (tt + 1)],
                                    t_bf[:], start=(k == 0),
                                    stop=(k == KH - 1))
                        gate_sb = []
                        for tt in range(TT):
                            g = gpool.tile([128, OBLK], f32, tag="gate")
                            nc.scalar.activation(g[:], ps_g[tt][:], Act.Silu,
                                                 scale=comb_g[tt][:])
                            gate_sb.append(g)
                        ps_u = [ppool.tile([128, OBLK], f32, tag="mm",
                                           name="ps_u") for _ in range(TT)]
                        for k in range(KH):
                            wti = wpool.tile([128, OBLK], f32, tag="w")
                            nc.sync.dma_start(
                                out=wti[:],
                                in_=wuT[128 * k:128 * (k + 1), osl])
                            t_i8 = tpool.tile([128, OBLK], i8, tag="ti8")
                            nc.scalar.activation(t_i8[:], wti[:], Act.Copy,
                                                 scale=inv_w_bc[:, 1:2])
                            t_bf = tpool.tile([128, OBLK], bf16, tag="tbf")
                            nc.vector.tensor_scalar(t_bf[:], t_i8[:], -1.0,
                                                    1.0, Alu.max, Alu.min)
                            for tt in range(TT):
                                nc.tensor.matmul(
                                    ps_u[tt][:],
                                    q1T[k][:, 128 * tt:128 * (tt + 1)],
                                    t_bf[:], start=(k == 0),
                                    stop=(k == KH - 1))
                        for tt in range(TT):
                            hs = hpool.tile([128, OBLK], f32, tag="hst")
                            nc.vector.scalar_tensor_tensor(
                                hs[:], ps_u[tt][:], comb_u[tt][:],
                                gate_sb[tt][:], Alu.mult, Alu.mult)
                            nc.sync.dma_start(
                                out=h2d[128 * tt:128 * (tt + 1), osl],
                                in_=hs[:])

            # ---------- phase 3: h -> [I,tok] fwht -> quant (transposed) --
            with tc.tile_pool(name="q2big", bufs=1) as q2bpool:
                Qq = [q2bpool.tile([128, I], bf16, tag=f"Qq{t}",
                                   name=f"Qq{t}") for t in range(TT)]
                with tc.tile_pool(name="hrow", bufs=1) as rpool, \
                     tc.tile_pool(name="ph3", bufs=1) as p3pool:
                    for tt in range(TT):
                        hrow = rpool.tile([128, I], f32, tag="hrow")
                        nc.sync.dma_start(out=hrow[:],
                                          in_=h2d[128 * tt:128 * (tt + 1)])
                        Q = p3pool.tile([128, I], f32, tag="Q")
                        Q2 = p3pool.tile([128, I], f32, tag="Q2")
                        # transpose 64 blocks: Q[b, 128a+t] = h[t, 128a+b]
                        for g in range(AI // 4):
                            ps = trpool.tile([128, 512], f32, tag="tr")
                            for s4 in range(4):
                                a = 4 * g + s4
                                nc.tensor.transpose(
                                    ps[:, 128 * s4:128 * (s4 + 1)],
                                    hrow[:, 128 * a:128 * (a + 1)],
                                    ident_f[:])
                            nc.scalar.activation(
                                Q[:, 512 * g:512 * (g + 1)], ps[:], Act.Copy)
                        # 6 cross-tile butterfly stages over a (v/g split)
                        cur, nxt = Q[:], Q2[:]
                        for s in (1, 2, 4, 8, 16, 32):
                            _butterfly_vg(nc, Alu, cur, nxt, 128 * s)
                            cur, nxt = nxt, cur
                        # cur is Q again (6 stages). H_128 over partitions
                        # via f32 PE matmuls; evacuate psum into Q2.
                        for c in range(I // 512):
                            psH = ppool.tile([128, 512], f32, tag="mm",
                                             name="psH")
                            nc.tensor.matmul(psH[:], h128_sb[:],
                                             cur[:, 512 * c:512 * (c + 1)],
                                             start=True, stop=True)
                            nc.scalar.activation(
                                Q2[:, 512 * c:512 * (c + 1)], psH[:],
                                Act.Copy)
                        # per-token absmax: reduce over a (strided), then
                        # over partitions via PE transpose + X-reduce.
                        q3 = Q2.rearrange("p (a t) -> p t a", a=AI)
                        tmax = spool.tile([128, 128], f32, tag="tmax")
                        nc.vector.tensor_reduce(tmax[:], q3, Axis.X, Alu.max,
                                                apply_absolute_value=True)
                        psT = trpool.tile([128, 128], f32, tag="tr")
                        nc.tensor.transpose(psT[:], tmax[:], ident_f[:])
                        amax = spool.tile([128, 1], f32, tag="amax2")
                        nc.vector.tensor_reduce(amax[:], psT[:], Axis.X,
                                                Alu.max,
                                                apply_absolute_value=True)
                        a_c = spool.tile([128, 1], f32, tag="ac2")
                        nc.vector.tensor_scalar(a_c[:], amax[:], NORM_I, EPS,
                                                Alu.mult, Alu.max)
                        ipost = spool.tile([128, 1], f32, tag="ip2")
                        nc.vector.tensor_scalar_mul(ipost[:], a_c[:],
                                                    1.0 / 127.0)
                        r2 = spool.tile([128, 1], f32, tag="r2")
                        nc.vector.reciprocal(r2[:], ipost[:])
                        qs_col = spool.tile([128, 1], f32, tag="qs2")
                        nc.vector.tensor_scalar_mul(qs_col[:], r2[:], NORM_I)
                        cd = spool.tile([128, 1], f32, tag=f"cd_{tt}")
                        nc.vector.tensor_tensor(cd[:], ipost[:],
                                                s_w_bc[:, 2:3], Alu.mult)
                        comb_d.append(cd)
                        # broadcast qs over a: transpose to a row, then
                        # ones (x) row outer product.
                        psR = trpool.tile([128, 128], f32, tag="tr")
                        nc.tensor.transpose(psR[0:1, :], qs_col[:],
                                            ident_f[:])
                        qs_row = spool.tile([1, 128], f32, tag="qsrow")
                        nc.vector.tensor_copy(qs_row[:], psR[0:1, :])
                        psB = trpool.tile([128, 128], f32, tag="tr")
                        nc.tensor.matmul(psB[:], ones_row[:], qs_row[:],
                                         start=True, stop=True)
                        qs_bc = spool.tile([128, 128], f32, tag="qsbc")
                        nc.vector.tensor_copy(qs_bc[:], psB[:])
                        # quantize: Q = Q2 * qs (broadcast over a), then
                        # rint+sat to i8 (scalar), then bf16 (vector).
                        q3o = Q.rearrange("p (a t) -> p a t", a=AI)
                        q3i = Q2.rearrange("p (a t) -> p a t", a=AI)
                        s3 = qs_bc.rearrange("p (o t) -> p o t", o=1)
                        b_in, b_s = broadcast_tensor_aps(q3i, s3)
                        nc.vector.tensor_tensor(q3o, b_in, b_s, Alu.mult)
                        qi8 = p3pool.tile([128, I], i8, tag="qi8")
                        nc.scalar.activation(qi8[:], Q[:], Act.Copy)
                        nc.vector.tensor_copy(Qq[tt][:], qi8[:])

                # ---------- phase 4: down matmul ----------
                with tc.tile_pool(name="wload4", bufs=8) as wpool, \
                     tc.tile_pool(name="tern4", bufs=8) as tpool, \
                     tc.tile_pool(name="osb", bufs=8) as opool:
                    for ob in range(NOB_D):
                        osl = slice(OBLK * ob, OBLK * (ob + 1))
                        ps_d = [ppool.tile([128, OBLK], f32, tag="mm",
                                           name="ps_d") for _ in range(TT)]
                        for k in range(KI):
                            wti = wpool.tile([128, OBLK], f32, tag="w")
                            nc.sync.dma_start(
                                out=wti[:],
                                in_=wdT[128 * k:128 * (k + 1), osl])
                            t_i8 = tpool.tile([128, OBLK], i8, tag="ti8")
                            nc.scalar.activation(t_i8[:], wti[:], Act.Copy,
                                                 scale=inv_w_bc[:, 2:3])
                            t_bf = tpool.tile([128, OBLK], bf16, tag="tbf")
                            nc.vector.tensor_scalar(t_bf[:], t_i8[:], -1.0,
                                                    1.0, Alu.max, Alu.min)
                            for tt in range(TT):
                                nc.tensor.matmul(
                                    ps_d[tt][:],
                                    Qq[tt][:, 128 * k:128 * (k + 1)],
                                    t_bf[:], start=(k == 0),
                                    stop=(k == KI - 1))
                        for tt in range(TT):
                            o_sb = opool.tile([128, OBLK], f32, tag="out")
                            nc.scalar.activation(o_sb[:], ps_d[tt][:],
                                                 Act.Copy,
                                                 scale=comb_d[tt][:])
                            nc.sync.dma_start(
                                out=out[128 * tt:128 * (tt + 1), osl],
                                in_=o_sb[:])

    nc.compile()
    return nc


def _get_program():
    if "nc" not in _CACHE:
        _CACHE["nc"] = _build_program()
    return _CACHE["nc"]


def _hadamard128():
    idx = np.arange(128, dtype=np.int64)
    anded = idx[:, None] & idx[None, :]
    pc = np.zeros_like(anded)
    for b in range(7):
        pc += (anded >> b) & 1
    return (1.0 - 2.0 * (pc % 2)).astype(np.float32)


def _make_in_maps(hidden_states, w_gate, w_up, w_down):
    x2 = np.ascontiguousarray(hidden_states.reshape(B * S, H),
                              dtype=np.float32)
    wgT = np.ascontiguousarray(np.asarray(w_gate, dtype=np.float32).T)
    wuT = np.ascontiguousarray(np.asarray(w_up, dtype=np.float32).T)
    wdT = np.ascontiguousarray(np.asarray(w_down, dtype=np.float32).T)
    h128 = _hadamard128()

    ci = I // 8
    ch = H // 8
    in_maps = [
        {
            "x": np.ascontiguousarray(x2[T * j:T * (j + 1)]),
            "wgT": wgT, "wuT": wuT, "wdT": wdT, "h128": h128,
            "wg_pre": np.ascontiguousarray(wgT[:, ci * j:ci * (j + 1)]),
            "wu_pre": np.ascontiguousarray(wuT[:, ci * j:ci * (j + 1)]),
            "wd_pre": np.ascontiguousarray(wdT[:, ch * j:ch * (j + 1)]),
        }
        for j in range(N_CORES)
    ]
    return in_maps


def kernel(hidden_states, w_gate, w_up, w_down, _trace=False):
    from concourse.bass_utils import run_bass_kernel_spmd

    nc = _get_program()
    in_maps = _make_in_maps(hidden_states, w_gate, w_up, w_down)
    res = run_bass_kernel_spmd(nc, in_maps, list(range(N_CORES)),
                               trace=_trace)
    pieces = [res.results[j]["out"] for j in range(N_CORES)]
    out = np.concatenate(pieces, axis=0).reshape(B, S, H)
    out = np.ascontiguousarray(out, dtype=np.float32)
    if _trace:
        return out, res
    return out


# revision 9
# speedup vs baseline: 1.1240x; 1.0163x over previous
"""BitNetV3 MLP kernel for 8 Trainium2 NeuronCores.

Data-parallel over tokens (8 x 512). v2: restructured for engine overlap.

Key changes vs v1 (1701us):
- Phase 0/1 reordered: x-FWHT runs on vector/gpsimd BEFORE the prepass
  reduces in queue order, so the weight-scale AllReduce latency hides
  under phase-1 compute. First matmul ~140us (was 239us).
- Phase 3 (FWHT over I=8192) rewritten: h rows are PE-transposed to
  [I_part, tok] layout first; the FWHT factorizes as
  H_64 (tile axis, 6 butterfly stages split vector/gpsimd) (x)
  H_128 (partition axis, exact f32 PE matmuls vs a +-1 Hadamard
  stationary). Per-token absmax/quant done in the transposed layout via
  strided reduce + PE row-broadcasts. Was 617us of vector-bound wall
  with PE idle; now ~4x less vector work in that stretch.
- Phase 4 consumes activation k-slices directly from the per-token-tile
  transposed bf16 tiles (no separate q2T copy step).

Exactness notes unchanged from v1: int8 casts are rint+sat so they
match jnp.round+clip; ternary weights and int8 activations are exact in
bf16; 1/scales fold into per-token post-scales; FWHT butterflies f32;
the H_128 stage uses the PE fp32 path (2-pass split, ~2^-16 relative)
which perturbs quantization decisions by well under the error budget.
"""

import sys
import numpy as np

if "/opt/trn_rl_repo" not in sys.path:
    sys.path.insert(0, "/opt/trn_rl_repo")

B, S, H, I = 2, 2048, 2048, 8192
N_CORES = 8
T = (B * S) // N_CORES  # tokens per core = 512
TT = T // 128           # token tiles per core = 4
EPS = 1e-5
NORM_H = float(1.0 / np.sqrt(H))
NORM_I = float(1.0 / np.sqrt(I))

KH = H // 128   # 16 k-tiles (gate/up contraction)
KI = I // 128   # 64 k-tiles (down contraction)
OBLK = 512
NOB_GU = I // OBLK   # 16
NOB_D = H // OBLK    # 4
AI = I // 128        # 64 column-blocks (a axis) in transposed layout

_CACHE = {}


def _butterfly(nc, Alu, src, dst, h):
    """One butterfly stage (pairs at stride h), all on vector."""
    ca = src.rearrange("p (n two h) -> p n two h", two=2, h=h)
    na = dst.rearrange("p (n two h) -> p n two h", two=2, h=h)
    a = ca[:, :, 0, :]
    b = ca[:, :, 1, :]
    nc.vector.tensor_tensor(na[:, :, 0, :], a, b, Alu.add)
    nc.vector.tensor_tensor(na[:, :, 1, :], a, b, Alu.subtract)


def _butterfly_vg(nc, Alu, src, dst, h):
    """One butterfly stage split ~3:1 across vector and gpsimd (gpsimd
    f32 elementwise is ~3.4x slower than vector, so it gets 1/4)."""
    ca = src.rearrange("p (n two h) -> p n two h", two=2, h=h)
    na = dst.rearrange("p (n two h) -> p n two h", two=2, h=h)
    hv = (3 * h) // 4
    for eng, sl in ((nc.vector, slice(0, hv)), (nc.gpsimd, slice(hv, h))):
        a = ca[:, :, 0, sl]
        b = ca[:, :, 1, sl]
        eng.tensor_tensor(na[:, :, 0, sl], a, b, Alu.add)
        eng.tensor_tensor(na[:, :, 1, sl], a, b, Alu.subtract)


def _build_program():
    import concourse.mybir as mybir
    from concourse import bacc
    from concourse.bass import broadcast_tensor_aps
    from concourse.tile import TileContext
    from concourse.masks import make_identity

    f32 = mybir.dt.float32
    bf16 = mybir.dt.bfloat16
    i8 = mybir.dt.int8
    Alu = mybir.AluOpType
    Act = mybir.ActivationFunctionType
    Axis = mybir.AxisListType

    nc = bacc.Bacc("TRN2", target_bir_lowering=False, debug=False,
                   num_devices=N_CORES)

    x = nc.dram_tensor("x", [T, H], f32, kind="ExternalInput").ap()
    wgT = nc.dram_tensor("wgT", [H, I], f32, kind="ExternalInput").ap()
    wuT = nc.dram_tensor("wuT", [H, I], f32, kind="ExternalInput").ap()
    wdT = nc.dram_tensor("wdT", [I, H], f32, kind="ExternalInput").ap()
    h128 = nc.dram_tensor("h128", [128, 128], f32, kind="ExternalInput").ap()
    # per-core 1/8 slices for the global-scale prepass
    wg_pre = nc.dram_tensor("wg_pre", [H, I // 8], f32, kind="ExternalInput").ap()
    wu_pre = nc.dram_tensor("wu_pre", [H, I // 8], f32, kind="ExternalInput").ap()
    wd_pre = nc.dram_tensor("wd_pre", [I, H // 8], f32, kind="ExternalInput").ap()
    out = nc.dram_tensor("out", [T, H], f32, kind="ExternalOutput").ap()

    h2d = nc.dram_tensor("h2d", [T, I], f32).ap()  # spilled h = silu(g)*u
    cc_in = nc.dram_tensor("cc_in", [1, 8], f32)
    cc_out = nc.dram_tensor("cc_out", [1, 8], f32, addr_space="Shared")

    with TileContext(nc) as tc:
        with (
            tc.tile_pool(name="const", bufs=1) as cpool,
            tc.tile_pool(name="scal", bufs=1) as spool,
            tc.tile_pool(name="psum", bufs=6, space="PSUM") as ppool,
            tc.tile_pool(name="psum_tr", bufs=2, space="PSUM") as trpool,
        ):
            # ---------------- constants ----------------
            ident_bf = cpool.tile([128, 128], bf16)
            make_identity(nc, ident_bf[:])
            ident_f = cpool.tile([128, 128], f32)
            make_identity(nc, ident_f[:])
            ones_row = cpool.tile([1, 128], f32)
            nc.vector.memset(ones_row[:], 1.0)
            h128_sb = cpool.tile([128, 128], f32)
            nc.sync.dma_start(out=h128_sb[:], in_=h128[:, :])

            comb_g, comb_u, comb_d = [], [], []

            # ---------------- phase 0: global weight scales ---------------
            st = spool.tile([1, 8], f32)
            nc.vector.memset(st[:], 0.0)
            with tc.tile_pool(name="pre", bufs=4) as prepool:
                def abs_sum_slice(wpre, idx):
                    rows, cols = wpre.shape
                    ntile = rows // 128
                    acc = spool.tile([128, 1], f32, tag=f"acc{idx}")
                    for k in range(ntile):
                        wtile = prepool.tile([128, cols], f32, tag="pre")
                        nc.sync.dma_start(out=wtile[:],
                                          in_=wpre[128 * k:128 * (k + 1)])
                        part = spool.tile([128, 1], f32, tag=f"part{idx}")
                        nc.vector.tensor_reduce(part[:], wtile[:], Axis.X,
                                                Alu.add,
                                                apply_absolute_value=True)
                        if k == 0:
                            nc.vector.tensor_copy(acc[:], part[:])
                        else:
                            nc.vector.tensor_tensor(acc[:], acc[:], part[:],
                                                    Alu.add)
                    tot = spool.tile([1, 1], f32, tag=f"tot{idx}")
                    nc.gpsimd.tensor_reduce(tot[:], acc[:], Axis.C, Alu.add)
                    nc.vector.tensor_copy(st[0:1, idx:idx + 1], tot[:])

                abs_sum_slice(wg_pre, 0)
                abs_sum_slice(wu_pre, 1)
                abs_sum_slice(wd_pre, 2)

            nc.sync.dma_start(out=cc_in[:], in_=st[:])
            nc.gpsimd.collective_compute(
                "AllReduce", Alu.add, ins=[cc_in[:]], outs=[cc_out[:]],
                replica_groups=[list(range(N_CORES))])
            sums = spool.tile([1, 8], f32)
            nc.sync.dma_start(out=sums[:], in_=cc_out[:])

            # s_w = max(mean, EPS); inv_s = 1/s_w
            means = spool.tile([1, 8], f32)
            nc.vector.tensor_scalar(means[:], sums[:], 1.0 / (H * I), EPS,
                                    Alu.mult, Alu.max)
            invs = spool.tile([1, 8], f32)
            nc.vector.reciprocal(invs[:], means[:])
            bc_ps = trpool.tile([128, 8], f32, tag="tr")
            nc.tensor.matmul(bc_ps[:], ones_row[:], means[:],
                             start=True, stop=True)
            s_w_bc = spool.tile([128, 8], f32)
            nc.vector.tensor_copy(s_w_bc[:], bc_ps[:])
            bc_ps2 = trpool.tile([128, 8], f32, tag="tr")
            nc.tensor.matmul(bc_ps2[:], ones_row[:], invs[:],
                             start=True, stop=True)
            inv_w_bc = spool.tile([128, 8], f32)
            nc.vector.tensor_copy(inv_w_bc[:], bc_ps2[:])

            with tc.tile_pool(name="q1T", bufs=1) as q1Tpool:
                q1T = [q1Tpool.tile([128, T], bf16, tag=f"q1T_{k}",
                                    name=f"q1T_{k}") for k in range(KH)]

                # ---------- phase 1: x -> fwht -> quant -> q1T ----------
                with tc.tile_pool(name="xb", bufs=2) as xpool, \
                     tc.tile_pool(name="q1s", bufs=2) as q1pool:
                    for tt in range(TT):
                        xa = xpool.tile([128, H], f32, tag="xa")
                        xb2 = xpool.tile([128, H], f32, tag="xb2")
                        nc.sync.dma_start(out=xa[:],
                                          in_=x[128 * tt:128 * (tt + 1)])
                        cur, nxt = xa[:], xb2[:]
                        h = 1
                        while h < H:
                            _butterfly(nc, Alu, cur, nxt, h)
                            cur, nxt = nxt, cur
                            h *= 2
                        amax = spool.tile([128, 1], f32, tag=f"amax1_{tt}")
                        nc.vector.tensor_reduce(amax[:], cur, Axis.X, Alu.max,
                                                apply_absolute_value=True)
                        a_c = spool.tile([128, 1], f32, tag=f"ac1_{tt}")
                        nc.vector.tensor_scalar(a_c[:], amax[:], NORM_H, EPS,
                                                Alu.mult, Alu.max)
                        ipost = spool.tile([128, 1], f32, tag=f"ip1_{tt}")
                        nc.vector.tensor_scalar_mul(ipost[:], a_c[:],
                                                    1.0 / 127.0)
                        r1 = spool.tile([128, 1], f32, tag=f"r1_{tt}")
                        nc.vector.reciprocal(r1[:], ipost[:])
                        qs = spool.tile([128, 1], f32, tag=f"qs1_{tt}")
                        nc.vector.tensor_scalar_mul(qs[:], r1[:], NORM_H)
                        cg = spool.tile([128, 1], f32, tag=f"cg_{tt}")
                        nc.vector.tensor_tensor(cg[:], ipost[:],
                                                s_w_bc[:, 0:1], Alu.mult)
                        comb_g.append(cg)
                        cu = spool.tile([128, 1], f32, tag=f"cu_{tt}")
                        nc.vector.tensor_tensor(cu[:], ipost[:],
                                                s_w_bc[:, 1:2], Alu.mult)
                        comb_u.append(cu)
                        q_i8 = q1pool.tile([128, H], i8, tag="q1i8")
                        nc.scalar.activation(q_i8[:], cur, Act.Copy,
                                             scale=qs[:])
                        q_bf = q1pool.tile([128, H], bf16, tag="q1bf")
                        nc.vector.tensor_copy(q_bf[:], q_i8[:])
                        for g in range(KH // 4):
                            ps = trpool.tile([128, 512], bf16, tag="tr")
                            for s4 in range(4):
                                k = 4 * g + s4
                                nc.tensor.transpose(
                                    ps[:, 128 * s4:128 * (s4 + 1)],
                                    q_bf[:, 128 * k:128 * (k + 1)],
                                    ident_bf[:])
                            for s4 in range(4):
                                k = 4 * g + s4
                                nc.vector.tensor_copy(
                                    q1T[k][:, 128 * tt:128 * (tt + 1)],
                                    ps[:, 128 * s4:128 * (s4 + 1)])

                # ---------- phase 2: gate/up matmuls + silu*up -> DRAM ----
                with tc.tile_pool(name="wload", bufs=8) as wpool, \
                     tc.tile_pool(name="tern", bufs=8) as tpool, \
                     tc.tile_pool(name="gsb", bufs=8) as gpool, \
                     tc.tile_pool(name="hst", bufs=8) as hpool:
                    for ob in range(NOB_GU):
                        osl = slice(OBLK * ob, OBLK * (ob + 1))
                        ps_g = [ppool.tile([128, OBLK], f32, tag="mm",
                                           name="ps_g") for _ in range(TT)]
                        for k in range(KH):
                            wti = wpool.tile([128, OBLK], f32, tag="w")
                            nc.sync.dma_start(
                                out=wti[:],
                                in_=wgT[128 * k:128 * (k + 1), osl])
                            t_i8 = tpool.tile([128, OBLK], i8, tag="ti8")
                            nc.scalar.activation(t_i8[:], wti[:], Act.Copy,
                                                 scale=inv_w_bc[:, 0:1])
                            t_bf = tpool.tile([128, OBLK], bf16, tag="tbf")
                            nc.vector.tensor_scalar(t_bf[:], t_i8[:], -1.0,
                                                    1.0, Alu.max, Alu.min)
                            for tt in range(TT):
                                nc.tensor.matmul(
                                    ps_g[tt][:],
                                    q1T[k][:, 128 * tt:128 * (tt + 1)],
                                    t_bf[:], start=(k == 0),
                                    stop=(k == KH - 1))
                        gate_sb = []
                        for tt in range(TT):
                            g = gpool.tile([128, OBLK], f32, tag="gate")
                            nc.scalar.activation(g[:], ps_g[tt][:], Act.Silu,
                                                 scale=comb_g[tt][:])
                            gate_sb.append(g)
                        ps_u = [ppool.tile([128, OBLK], f32, tag="mm",
                                           name="ps_u") for _ in range(TT)]
                        for k in range(KH):
                            wti = wpool.tile([128, OBLK], f32, tag="w")
                            nc.sync.dma_start(
                                out=wti[:],
                                in_=wuT[128 * k:128 * (k + 1), osl])
                            t_i8 = tpool.tile([128, OBLK], i8, tag="ti8")
                            nc.scalar.activation(t_i8[:], wti[:], Act.Copy,
                                                 scale=inv_w_bc[:, 1:2])
                            t_bf = tpool.tile([128, OBLK], bf16, tag="tbf")
                            nc.vector.tensor_scalar(t_bf[:], t_i8[:], -1.0,
                                                    1.0, Alu.max, Alu.min)
                            for tt in range(TT):
                                nc.tensor.matmul(
                                    ps_u[tt][:],
                                    q1T[k][:, 128 * tt:128 * (tt + 1)],
                                    t_bf[:], start=(k == 0),
                                    stop=(k == KH - 1))
                        for tt in range(TT):
                            hs = hpool.tile([128, OBLK], f32, tag="hst")
                            nc.vector.scalar_tensor_tensor(
                                hs[:], ps_u[tt][:], comb_u[tt][:],
                                gate_sb[tt][:], Alu.mult, Alu.mult)
                            nc.sync.dma_start(
                                out=h2d[128 * tt:128 * (tt + 1), osl],
                                in_=hs[:])

            # ---------- phase 3: h -> [I,tok] fwht -> quant (transposed) --
            with tc.tile_pool(name="q2big", bufs=1) as q2bpool:
                Qq = [q2bpool.tile([128, I], bf16, tag=f"Qq{t}",
                                   name=f"Qq{t}") for t in range(TT)]
                with tc.tile_pool(name="hrow", bufs=1) as rpool, \
                     tc.tile_pool(name="ph3", bufs=1) as p3pool:
                    for tt in range(TT):
                        hrow = rpool.tile([128, I], f32, tag="hrow")
                        nc.sync.dma_start(out=hrow[:],
                                          in_=h2d[128 * tt:128 * (tt + 1)])
                        Q = p3pool.tile([128, I], f32, tag="Q")
                        Q2 = p3pool.tile([128, I], f32, tag="Q2")
                        # transpose 64 blocks: Q[b, 128a+t] = h[t, 128a+b]
                        for g in range(AI // 4):
                            ps = trpool.tile([128, 512], f32, tag="tr")
                            for s4 in range(4):
                                a = 4 * g + s4
                                nc.tensor.transpose(
                                    ps[:, 128 * s4:128 * (s4 + 1)],
                                    hrow[:, 128 * a:128 * (a + 1)],
                                    ident_f[:])
                            nc.scalar.activation(
                                Q[:, 512 * g:512 * (g + 1)], ps[:], Act.Copy)
                        # 6 cross-tile butterfly stages over a (vector;
                        # gpsimd measured ~5ns/elem f32 — not worth it)
                        cur, nxt = Q[:], Q2[:]
                        for s in (1, 2, 4, 8, 16, 32):
                            _butterfly(nc, Alu, cur, nxt, 128 * s)
                            cur, nxt = nxt, cur
                        # cur is Q again (6 stages). H_128 over partitions
                        # via f32 PE matmuls; evacuate psum into Q2.
                        for c in range(I // 512):
                            psH = ppool.tile([128, 512], f32, tag="mm",
                                             name="psH")
                            nc.tensor.matmul(psH[:], h128_sb[:],
                                             cur[:, 512 * c:512 * (c + 1)],
                                             start=True, stop=True)
                            nc.scalar.activation(
                                Q2[:, 512 * c:512 * (c + 1)], psH[:],
                                Act.Copy)
                        # per-token absmax: reduce over a (strided), then
                        # over partitions via PE transpose + X-reduce.
                        q3 = Q2.rearrange("p (a t) -> p t a", a=AI)
                        tmax = spool.tile([128, 128], f32, tag="tmax")
                        nc.vector.tensor_reduce(tmax[:], q3, Axis.X, Alu.max,
                                                apply_absolute_value=True)
                        psT = trpool.tile([128, 128], f32, tag="tr")
                        nc.tensor.transpose(psT[:], tmax[:], ident_f[:])
                        amax = spool.tile([128, 1], f32, tag="amax2")
                        nc.vector.tensor_reduce(amax[:], psT[:], Axis.X,
                                                Alu.max,
                                                apply_absolute_value=True)
                        a_c = spool.tile([128, 1], f32, tag="ac2")
                        nc.vector.tensor_scalar(a_c[:], amax[:], NORM_I, EPS,
                                                Alu.mult, Alu.max)
                        ipost = spool.tile([128, 1], f32, tag="ip2")
                        nc.vector.tensor_scalar_mul(ipost[:], a_c[:],
                                                    1.0 / 127.0)
                        r2 = spool.tile([128, 1], f32, tag="r2")
                        nc.vector.reciprocal(r2[:], ipost[:])
                        qs_col = spool.tile([128, 1], f32, tag="qs2")
                        nc.vector.tensor_scalar_mul(qs_col[:], r2[:], NORM_I)
                        cd = spool.tile([128, 1], f32, tag=f"cd_{tt}")
                        nc.vector.tensor_tensor(cd[:], ipost[:],
                                                s_w_bc[:, 2:3], Alu.mult)
                        comb_d.append(cd)
                        # broadcast qs over a: transpose to a row, then
                        # ones (x) row outer product.
                        psR = trpool.tile([128, 128], f32, tag="tr")
                        nc.tensor.transpose(psR[0:1, :], qs_col[:],
                                            ident_f[:])
                        qs_row = spool.tile([1, 128], f32, tag="qsrow")
                        nc.vector.tensor_copy(qs_row[:], psR[0:1, :])
                        psB = trpool.tile([128, 128], f32, tag="tr")
                        nc.tensor.matmul(psB[:], ones_row[:], qs_row[:],
                                         start=True, stop=True)
                        qs_bc = spool.tile([128, 128], f32, tag="qsbc")
                        nc.vector.tensor_copy(qs_bc[:], psB[:])
                        # quantize: Q = Q2 * qs (broadcast over a), then
                        # rint+sat to i8 (scalar), then bf16 (vector).
                        q3o = Q.rearrange("p (a t) -> p a t", a=AI)
                        q3i = Q2.rearrange("p (a t) -> p a t", a=AI)
                        s3 = qs_bc.rearrange("p (o t) -> p o t", o=1)
                        b_in, b_s = broadcast_tensor_aps(q3i, s3)
                        nc.vector.tensor_tensor(q3o, b_in, b_s, Alu.mult)
                        qi8 = p3pool.tile([128, I], i8, tag="qi8")
                        nc.scalar.activation(qi8[:], Q[:], Act.Copy)
                        nc.vector.tensor_copy(Qq[tt][:], qi8[:])

                # ---------- phase 4: down matmul ----------
                with tc.tile_pool(name="wload4", bufs=12) as wpool, \
                     tc.tile_pool(name="tern4", bufs=10) as tpool, \
                     tc.tile_pool(name="osb", bufs=8) as opool:
                    for ob in range(NOB_D):
                        osl = slice(OBLK * ob, OBLK * (ob + 1))
                        ps_d = [ppool.tile([128, OBLK], f32, tag="mm",
                                           name="ps_d") for _ in range(TT)]
                        for k in range(KI):
                            wti = wpool.tile([128, OBLK], f32, tag="w")
                            nc.sync.dma_start(
                                out=wti[:],
                                in_=wdT[128 * k:128 * (k + 1), osl])
                            t_i8 = tpool.tile([128, OBLK], i8, tag="ti8")
                            nc.scalar.activation(t_i8[:], wti[:], Act.Copy,
                                                 scale=inv_w_bc[:, 2:3])
                            t_bf = tpool.tile([128, OBLK], bf16, tag="tbf")
                            nc.vector.tensor_scalar(t_bf[:], t_i8[:], -1.0,
                                                    1.0, Alu.max, Alu.min)
                            for tt in range(TT):
                                nc.tensor.matmul(
                                    ps_d[tt][:],
                                    Qq[tt][:, 128 * k:128 * (k + 1)],
                                    t_bf[:], start=(k == 0),
                                    stop=(k == KI - 1))
                        for tt in range(TT):
                            o_sb = opool.tile([128, OBLK], f32, tag="out")
                            nc.scalar.activation(o_sb[:], ps_d[tt][:],
                                                 Act.Copy,
                                                 scale=comb_d[tt][:])
                            nc.sync.dma_start(
                                out=out[128 * tt:128 * (tt + 1), osl],
                                in_=o_sb[:])

    nc.compile()
    return nc


def _get_program():
    if "nc" not in _CACHE:
        _CACHE["nc"] = _build_program()
    return _CACHE["nc"]


def _hadamard128():
    idx = np.arange(128, dtype=np.int64)
    anded = idx[:, None] & idx[None, :]
    pc = np.zeros_like(anded)
    for b in range(7):
        pc += (anded >> b) & 1
    return (1.0 - 2.0 * (pc % 2)).astype(np.float32)


def _make_in_maps(hidden_states, w_gate, w_up, w_down):
    x2 = np.ascontiguousarray(hidden_states.reshape(B * S, H),
                              dtype=np.float32)
    wgT = np.ascontiguousarray(np.asarray(w_gate, dtype=np.float32).T)
    wuT = np.ascontiguousarray(np.asarray(w_up, dtype=np.float32).T)
    wdT = np.ascontiguousarray(np.asarray(w_down, dtype=np.float32).T)
    h128 = _hadamard128()

    ci = I // 8
    ch = H // 8
    in_maps = [
        {
            "x": np.ascontiguousarray(x2[T * j:T * (j + 1)]),
            "wgT": wgT, "wuT": wuT, "wdT": wdT, "h128": h128,
            "wg_pre": np.ascontiguousarray(wgT[:, ci * j:ci * (j + 1)]),
            "wu_pre": np.ascontiguousarray(wuT[:, ci * j:ci * (j + 1)]),
            "wd_pre": np.ascontiguousarray(wdT[:, ch * j:ch * (j + 1)]),
        }
        for j in range(N_CORES)
    ]
    return in_maps


def kernel(hidden_states, w_gate, w_up, w_down, _trace=False):
    from concourse.bass_utils import run_bass_kernel_spmd

    nc = _get_program()
    in_maps = _make_in_maps(hidden_states, w_gate, w_up, w_down)
    res = run_bass_kernel_spmd(nc, in_maps, list(range(N_CORES)),
                               trace=_trace)
    pieces = [res.results[j]["out"] for j in range(N_CORES)]
    out = np.concatenate(pieces, axis=0).reshape(B, S, H)
    out = np.ascontiguousarray(out, dtype=np.float32)
    if _trace:
        return out, res
    return out


# revision 10
# speedup vs baseline: 1.1637x; 1.0353x over previous
"""BitNetV3 MLP kernel for 8 Trainium2 NeuronCores.

Data-parallel over tokens (8 x 512). v2: restructured for engine overlap.

Key changes vs v1 (1701us):
- Phase 0/1 reordered: x-FWHT runs on vector/gpsimd BEFORE the prepass
  reduces in queue order, so the weight-scale AllReduce latency hides
  under phase-1 compute. First matmul ~140us (was 239us).
- Phase 3 (FWHT over I=8192) rewritten: h rows are PE-transposed to
  [I_part, tok] layout first; the FWHT factorizes as
  H_64 (tile axis, 6 butterfly stages split vector/gpsimd) (x)
  H_128 (partition axis, exact f32 PE matmuls vs a +-1 Hadamard
  stationary). Per-token absmax/quant done in the transposed layout via
  strided reduce + PE row-broadcasts. Was 617us of vector-bound wall
  with PE idle; now ~4x less vector work in that stretch.
- Phase 4 consumes activation k-slices directly from the per-token-tile
  transposed bf16 tiles (no separate q2T copy step).

Exactness notes unchanged from v1: int8 casts are rint+sat so they
match jnp.round+clip; ternary weights and int8 activations are exact in
bf16; 1/scales fold into per-token post-scales; FWHT butterflies f32;
the H_128 stage uses the PE fp32 path (2-pass split, ~2^-16 relative)
which perturbs quantization decisions by well under the error budget.
"""

import sys
import numpy as np

if "/opt/trn_rl_repo" not in sys.path:
    sys.path.insert(0, "/opt/trn_rl_repo")

B, S, H, I = 2, 2048, 2048, 8192
N_CORES = 8
T = (B * S) // N_CORES  # tokens per core = 512
TT = T // 128           # token tiles per core = 4
EPS = 1e-5
NORM_H = float(1.0 / np.sqrt(H))
NORM_I = float(1.0 / np.sqrt(I))

KH = H // 128   # 16 k-tiles (gate/up contraction)
KI = I // 128   # 64 k-tiles (down contraction)
OBLK = 512
NOB_GU = I // OBLK   # 16
NOB_D = H // OBLK    # 4
AI = I // 128        # 64 column-blocks (a axis) in transposed layout

_CACHE = {}


def _butterfly(nc, Alu, src, dst, h):
    """One butterfly stage (pairs at stride h), all on vector."""
    ca = src.rearrange("p (n two h) -> p n two h", two=2, h=h)
    na = dst.rearrange("p (n two h) -> p n two h", two=2, h=h)
    a = ca[:, :, 0, :]
    b = ca[:, :, 1, :]
    nc.vector.tensor_tensor(na[:, :, 0, :], a, b, Alu.add)
    nc.vector.tensor_tensor(na[:, :, 1, :], a, b, Alu.subtract)


def _butterfly_vg(nc, Alu, src, dst, h):
    """One butterfly stage split ~3:1 across vector and gpsimd (gpsimd
    f32 elementwise is ~3.4x slower than vector, so it gets 1/4)."""
    ca = src.rearrange("p (n two h) -> p n two h", two=2, h=h)
    na = dst.rearrange("p (n two h) -> p n two h", two=2, h=h)
    hv = (3 * h) // 4
    for eng, sl in ((nc.vector, slice(0, hv)), (nc.gpsimd, slice(hv, h))):
        a = ca[:, :, 0, sl]
        b = ca[:, :, 1, sl]
        eng.tensor_tensor(na[:, :, 0, sl], a, b, Alu.add)
        eng.tensor_tensor(na[:, :, 1, sl], a, b, Alu.subtract)


def _build_program():
    import concourse.mybir as mybir
    from concourse import bacc
    from concourse.bass import broadcast_tensor_aps
    from concourse.tile import TileContext
    from concourse.masks import make_identity

    f32 = mybir.dt.float32
    bf16 = mybir.dt.bfloat16
    i8 = mybir.dt.int8
    Alu = mybir.AluOpType
    Act = mybir.ActivationFunctionType
    Axis = mybir.AxisListType

    nc = bacc.Bacc("TRN2", target_bir_lowering=False, debug=False,
                   num_devices=N_CORES)

    x = nc.dram_tensor("x", [T, H], f32, kind="ExternalInput").ap()
    wgT = nc.dram_tensor("wgT", [H, I], f32, kind="ExternalInput").ap()
    wuT = nc.dram_tensor("wuT", [H, I], f32, kind="ExternalInput").ap()
    wdT = nc.dram_tensor("wdT", [I, H], f32, kind="ExternalInput").ap()
    h128 = nc.dram_tensor("h128", [128, 128], f32, kind="ExternalInput").ap()
    # per-core 1/8 slices for the global-scale prepass
    wg_pre = nc.dram_tensor("wg_pre", [H, I // 8], f32, kind="ExternalInput").ap()
    wu_pre = nc.dram_tensor("wu_pre", [H, I // 8], f32, kind="ExternalInput").ap()
    wd_pre = nc.dram_tensor("wd_pre", [I, H // 8], f32, kind="ExternalInput").ap()
    out = nc.dram_tensor("out", [T, H], f32, kind="ExternalOutput").ap()

    h2d = nc.dram_tensor("h2d", [T, I], f32).ap()  # spilled h = silu(g)*u
    cc_in = nc.dram_tensor("cc_in", [1, 8], f32)
    cc_out = nc.dram_tensor("cc_out", [1, 8], f32, addr_space="Shared")

    with TileContext(nc) as tc:
        with (
            tc.tile_pool(name="const", bufs=1) as cpool,
            tc.tile_pool(name="scal", bufs=1) as spool,
            tc.tile_pool(name="psum", bufs=6, space="PSUM") as ppool,
            tc.tile_pool(name="psum_tr", bufs=2, space="PSUM") as trpool,
        ):
            # ---------------- constants ----------------
            ident_bf = cpool.tile([128, 128], bf16)
            make_identity(nc, ident_bf[:])
            ident_f = cpool.tile([128, 128], f32)
            make_identity(nc, ident_f[:])
            ones_row = cpool.tile([1, 128], f32)
            nc.vector.memset(ones_row[:], 1.0)
            h128_sb = cpool.tile([128, 128], f32)
            nc.sync.dma_start(out=h128_sb[:], in_=h128[:, :])

            comb_g, comb_u, comb_d = [], [], []

            # ---------------- phase 0: global weight scales ---------------
            st = spool.tile([1, 8], f32)
            nc.vector.memset(st[:], 0.0)
            with tc.tile_pool(name="pre", bufs=4) as prepool:
                def abs_sum_slice(wpre, idx):
                    rows, cols = wpre.shape
                    ntile = rows // 128
                    acc = spool.tile([128, 1], f32, tag=f"acc{idx}")
                    for k in range(ntile):
                        wtile = prepool.tile([128, cols], f32, tag="pre")
                        nc.sync.dma_start(out=wtile[:],
                                          in_=wpre[128 * k:128 * (k + 1)])
                        part = spool.tile([128, 1], f32, tag=f"part{idx}")
                        nc.vector.tensor_reduce(part[:], wtile[:], Axis.X,
                                                Alu.add,
                                                apply_absolute_value=True)
                        if k == 0:
                            nc.vector.tensor_copy(acc[:], part[:])
                        else:
                            nc.vector.tensor_tensor(acc[:], acc[:], part[:],
                                                    Alu.add)
                    tot = spool.tile([1, 1], f32, tag=f"tot{idx}")
                    nc.gpsimd.tensor_reduce(tot[:], acc[:], Axis.C, Alu.add)
                    nc.vector.tensor_copy(st[0:1, idx:idx + 1], tot[:])

                abs_sum_slice(wg_pre, 0)
                abs_sum_slice(wu_pre, 1)
                abs_sum_slice(wd_pre, 2)

            nc.sync.dma_start(out=cc_in[:], in_=st[:])
            nc.gpsimd.collective_compute(
                "AllReduce", Alu.add, ins=[cc_in[:]], outs=[cc_out[:]],
                replica_groups=[list(range(N_CORES))])
            sums = spool.tile([1, 8], f32)
            nc.sync.dma_start(out=sums[:], in_=cc_out[:])

            # s_w = max(mean, EPS); inv_s = 1/s_w
            means = spool.tile([1, 8], f32)
            nc.vector.tensor_scalar(means[:], sums[:], 1.0 / (H * I), EPS,
                                    Alu.mult, Alu.max)
            invs = spool.tile([1, 8], f32)
            nc.vector.reciprocal(invs[:], means[:])
            bc_ps = trpool.tile([128, 8], f32, tag="tr")
            nc.tensor.matmul(bc_ps[:], ones_row[:], means[:],
                             start=True, stop=True)
            s_w_bc = spool.tile([128, 8], f32)
            nc.vector.tensor_copy(s_w_bc[:], bc_ps[:])
            bc_ps2 = trpool.tile([128, 8], f32, tag="tr")
            nc.tensor.matmul(bc_ps2[:], ones_row[:], invs[:],
                             start=True, stop=True)
            inv_w_bc = spool.tile([128, 8], f32)
            nc.vector.tensor_copy(inv_w_bc[:], bc_ps2[:])

            with tc.tile_pool(name="q1T", bufs=1) as q1Tpool:
                q1T = [q1Tpool.tile([128, T], bf16, tag=f"q1T_{k}",
                                    name=f"q1T_{k}") for k in range(KH)]

                # ---------- phase 1: x -> fwht -> quant -> q1T ----------
                with tc.tile_pool(name="xb", bufs=2) as xpool, \
                     tc.tile_pool(name="q1s", bufs=2) as q1pool:
                    for tt in range(TT):
                        xa = xpool.tile([128, H], f32, tag="xa")
                        xb2 = xpool.tile([128, H], f32, tag="xb2")
                        nc.sync.dma_start(out=xa[:],
                                          in_=x[128 * tt:128 * (tt + 1)])
                        cur, nxt = xa[:], xb2[:]
                        h = 1
                        while h < H:
                            _butterfly(nc, Alu, cur, nxt, h)
                            cur, nxt = nxt, cur
                            h *= 2
                        amax = spool.tile([128, 1], f32, tag=f"amax1_{tt}")
                        nc.vector.tensor_reduce(amax[:], cur, Axis.X, Alu.max,
                                                apply_absolute_value=True)
                        a_c = spool.tile([128, 1], f32, tag=f"ac1_{tt}")
                        nc.vector.tensor_scalar(a_c[:], amax[:], NORM_H, EPS,
                                                Alu.mult, Alu.max)
                        ipost = spool.tile([128, 1], f32, tag=f"ip1_{tt}")
                        nc.vector.tensor_scalar_mul(ipost[:], a_c[:],
                                                    1.0 / 127.0)
                        r1 = spool.tile([128, 1], f32, tag=f"r1_{tt}")
                        nc.vector.reciprocal(r1[:], ipost[:])
                        qs = spool.tile([128, 1], f32, tag=f"qs1_{tt}")
                        nc.vector.tensor_scalar_mul(qs[:], r1[:], NORM_H)
                        cg = spool.tile([128, 1], f32, tag=f"cg_{tt}")
                        nc.vector.tensor_tensor(cg[:], ipost[:],
                                                s_w_bc[:, 0:1], Alu.mult)
                        comb_g.append(cg)
                        cu = spool.tile([128, 1], f32, tag=f"cu_{tt}")
                        nc.vector.tensor_tensor(cu[:], ipost[:],
                                                s_w_bc[:, 1:2], Alu.mult)
                        comb_u.append(cu)
                        q_i8 = q1pool.tile([128, H], i8, tag="q1i8")
                        nc.scalar.activation(q_i8[:], cur, Act.Copy,
                                             scale=qs[:])
                        q_bf = q1pool.tile([128, H], bf16, tag="q1bf")
                        nc.vector.tensor_copy(q_bf[:], q_i8[:])
                        for g in range(KH // 4):
                            ps = trpool.tile([128, 512], bf16, tag="tr")
                            for s4 in range(4):
                                k = 4 * g + s4
                                nc.tensor.transpose(
                                    ps[:, 128 * s4:128 * (s4 + 1)],
                                    q_bf[:, 128 * k:128 * (k + 1)],
                                    ident_bf[:])
                            for s4 in range(4):
                                k = 4 * g + s4
                                nc.vector.tensor_copy(
                                    q1T[k][:, 128 * tt:128 * (tt + 1)],
                                    ps[:, 128 * s4:128 * (s4 + 1)])

                # ---------- phase 2: gate/up matmuls + silu*up -> DRAM ----
                with tc.tile_pool(name="wload", bufs=8) as wpool, \
                     tc.tile_pool(name="tern", bufs=8) as tpool, \
                     tc.tile_pool(name="gsb", bufs=8) as gpool, \
                     tc.tile_pool(name="hst", bufs=8) as hpool:
                    for ob in range(NOB_GU):
                        osl = slice(OBLK * ob, OBLK * (ob + 1))
                        ps_g = [ppool.tile([128, OBLK], f32, tag="mm",
                                           name="ps_g") for _ in range(TT)]
                        for k in range(KH):
                            wti = wpool.tile([128, OBLK], f32, tag="w")
                            nc.sync.dma_start(
                                out=wti[:],
                                in_=wgT[128 * k:128 * (k + 1), osl])
                            t_i8 = tpool.tile([128, OBLK], i8, tag="ti8")
                            nc.scalar.activation(t_i8[:], wti[:], Act.Copy,
                                                 scale=inv_w_bc[:, 0:1])
                            t_bf = tpool.tile([128, OBLK], bf16, tag="tbf")
                            nc.vector.tensor_scalar(t_bf[:], t_i8[:], -1.0,
                                                    1.0, Alu.max, Alu.min)
                            for tt in range(TT):
                                nc.tensor.matmul(
                                    ps_g[tt][:],
                                    q1T[k][:, 128 * tt:128 * (tt + 1)],
                                    t_bf[:], start=(k == 0),
                                    stop=(k == KH - 1))
                        gate_sb = []
                        for tt in range(TT):
                            g = gpool.tile([128, OBLK], f32, tag="gate")
                            nc.scalar.activation(g[:], ps_g[tt][:], Act.Silu,
                                                 scale=comb_g[tt][:])
                            gate_sb.append(g)
                        ps_u = [ppool.tile([128, OBLK], f32, tag="mm",
                                           name="ps_u") for _ in range(TT)]
                        for k in range(KH):
                            wti = wpool.tile([128, OBLK], f32, tag="w")
                            nc.sync.dma_start(
                                out=wti[:],
                                in_=wuT[128 * k:128 * (k + 1), osl])
                            t_i8 = tpool.tile([128, OBLK], i8, tag="ti8")
                            nc.scalar.activation(t_i8[:], wti[:], Act.Copy,
                                                 scale=inv_w_bc[:, 1:2])
                            t_bf = tpool.tile([128, OBLK], bf16, tag="tbf")
                            nc.vector.tensor_scalar(t_bf[:], t_i8[:], -1.0,
                                                    1.0, Alu.max, Alu.min)
                            for tt in range(TT):
                                nc.tensor.matmul(
                                    ps_u[tt][:],
                                    q1T[k][:, 128 * tt:128 * (tt + 1)],
                                    t_bf[:], start=(k == 0),
                                    stop=(k == KH - 1))
                        for tt in range(TT):
                            hs = hpool.tile([128, OBLK], f32, tag="hst")
                            nc.vector.scalar_tensor_tensor(
                                hs[:], ps_u[tt][:], comb_u[tt][:],
                                gate_sb[tt][:], Alu.mult, Alu.mult)
                            nc.sync.dma_start(
                                out=h2d[128 * tt:128 * (tt + 1), osl],
                                in_=hs[:])

            # ---------- phase 3: h -> [I,tok] fwht -> quant (transposed) --
            with tc.tile_pool(name="q2big", bufs=1) as q2bpool:
                Qq = [q2bpool.tile([128, I], bf16, tag=f"Qq{t}",
                                   name=f"Qq{t}") for t in range(TT)]
                with tc.tile_pool(name="hrow", bufs=1) as rpool, \
                     tc.tile_pool(name="ph3", bufs=1) as p3pool:
                    for tt in range(TT):
                        hrow = rpool.tile([128, I], f32, tag="hrow")
                        nc.sync.dma_start(out=hrow[:],
                                          in_=h2d[128 * tt:128 * (tt + 1)])
                        Q = p3pool.tile([128, I], f32, tag="Q")
                        Q2 = p3pool.tile([128, I], f32, tag="Q2")
                        # transpose 64 blocks: Q[b, 128a+t] = h[t, 128a+b]
                        for g in range(AI // 4):
                            ps = trpool.tile([128, 512], f32, tag="tr")
                            for s4 in range(4):
                                a = 4 * g + s4
                                nc.tensor.transpose(
                                    ps[:, 128 * s4:128 * (s4 + 1)],
                                    hrow[:, 128 * a:128 * (a + 1)],
                                    ident_f[:])
                            nc.scalar.activation(
                                Q[:, 512 * g:512 * (g + 1)], ps[:], Act.Copy)
                        # 6 cross-tile butterfly stages over a (vector;
                        # gpsimd measured ~5ns/elem f32 — not worth it)
                        cur, nxt = Q[:], Q2[:]
                        for s in (1, 2, 4, 8, 16, 32):
                            _butterfly(nc, Alu, cur, nxt, 128 * s)
                            cur, nxt = nxt, cur
                        # cur is Q again (6 stages). H_128 over partitions
                        # via f32 PE matmuls; evacuate psum into Q2.
                        for c in range(I // 512):
                            psH = ppool.tile([128, 512], f32, tag="mm",
                                             name="psH")
                            nc.tensor.matmul(psH[:], h128_sb[:],
                                             cur[:, 512 * c:512 * (c + 1)],
                                             start=True, stop=True)
                            nc.scalar.activation(
                                Q2[:, 512 * c:512 * (c + 1)], psH[:],
                                Act.Copy)
                        # per-token absmax: reduce over a (strided), then
                        # over partitions via PE transpose + X-reduce.
                        q3 = Q2.rearrange("p (a t) -> p t a", a=AI)
                        tmax = spool.tile([128, 128], f32, tag="tmax")
                        nc.vector.tensor_reduce(tmax[:], q3, Axis.X, Alu.max,
                                                apply_absolute_value=True)
                        psT = trpool.tile([128, 128], f32, tag="tr")
                        nc.tensor.transpose(psT[:], tmax[:], ident_f[:])
                        amax = spool.tile([128, 1], f32, tag="amax2")
                        nc.vector.tensor_reduce(amax[:], psT[:], Axis.X,
                                                Alu.max,
                                                apply_absolute_value=True)
                        a_c = spool.tile([128, 1], f32, tag="ac2")
                        nc.vector.tensor_scalar(a_c[:], amax[:], NORM_I, EPS,
                                                Alu.mult, Alu.max)
                        ipost = spool.tile([128, 1], f32, tag="ip2")
                        nc.vector.tensor_scalar_mul(ipost[:], a_c[:],
                                                    1.0 / 127.0)
                        r2 = spool.tile([128, 1], f32, tag="r2")
                        nc.vector.reciprocal(r2[:], ipost[:])
                        qs_col = spool.tile([128, 1], f32, tag="qs2")
                        nc.vector.tensor_scalar_mul(qs_col[:], r2[:], NORM_I)
                        cd = spool.tile([128, 1], f32, tag=f"cd_{tt}")
                        nc.vector.tensor_tensor(cd[:], ipost[:],
                                                s_w_bc[:, 2:3], Alu.mult)
                        comb_d.append(cd)
                        # broadcast qs over a: transpose to a row, then
                        # ones (x) row outer product.
                        psR = trpool.tile([128, 128], f32, tag="tr")
                        nc.tensor.transpose(psR[0:1, :], qs_col[:],
                                            ident_f[:])
                        qs_row = spool.tile([1, 128], f32, tag="qsrow")
                        nc.vector.tensor_copy(qs_row[:], psR[0:1, :])
                        psB = trpool.tile([128, 128], f32, tag="tr")
                        nc.tensor.matmul(psB[:], ones_row[:], qs_row[:],
                                         start=True, stop=True)
                        qs_bc = spool.tile([128, 128], f32, tag="qsbc")
                        nc.vector.tensor_copy(qs_bc[:], psB[:])
                        # quantize: Q2 *= qs in place (broadcast over a;
                        # elementwise same-offset is safe on DVE), then
                        # rint+sat to i8 (scalar), then bf16 (vector).
                        # Keeping the multiply out of Q makes the H-matmul
                        # the last reader of Q, so the next tile's
                        # transposes overlap this tile's quant tail.
                        q3i = Q2.rearrange("p (a t) -> p a t", a=AI)
                        s3 = qs_bc.rearrange("p (o t) -> p o t", o=1)
                        b_in, b_s = broadcast_tensor_aps(q3i, s3)
                        nc.vector.tensor_tensor(q3i, b_in, b_s, Alu.mult)
                        qi8 = p3pool.tile([128, I], i8, tag="qi8")
                        nc.scalar.activation(qi8[:], Q2[:], Act.Copy)
                        nc.vector.tensor_copy(Qq[tt][:], qi8[:])

                # ---------- phase 4: down matmul ----------
                with tc.tile_pool(name="wload4", bufs=12) as wpool, \
                     tc.tile_pool(name="tern4", bufs=10) as tpool, \
                     tc.tile_pool(name="osb", bufs=8) as opool:
                    for ob in range(NOB_D):
                        osl = slice(OBLK * ob, OBLK * (ob + 1))
                        ps_d = [ppool.tile([128, OBLK], f32, tag="mm",
                                           name="ps_d") for _ in range(TT)]
                        for k in range(KI):
                            wti = wpool.tile([128, OBLK], f32, tag="w")
                            nc.sync.dma_start(
                                out=wti[:],
                                in_=wdT[128 * k:128 * (k + 1), osl])
                            t_i8 = tpool.tile([128, OBLK], i8, tag="ti8")
                            nc.scalar.activation(t_i8[:], wti[:], Act.Copy,
                                                 scale=inv_w_bc[:, 2:3])
                            t_bf = tpool.tile([128, OBLK], bf16, tag="tbf")
                            nc.vector.tensor_scalar(t_bf[:], t_i8[:], -1.0,
                                                    1.0, Alu.max, Alu.min)
                            for tt in range(TT):
                                nc.tensor.matmul(
                                    ps_d[tt][:],
                                    Qq[tt][:, 128 * k:128 * (k + 1)],
                                    t_bf[:], start=(k == 0),
                                    stop=(k == KI - 1))
                        for tt in range(TT):
                            o_sb = opool.tile([128, OBLK], f32, tag="out")
                            nc.scalar.activation(o_sb[:], ps_d[tt][:],
                                                 Act.Copy,
                                                 scale=comb_d[tt][:])
                            nc.sync.dma_start(
                                out=out[128 * tt:128 * (tt + 1), osl],
                                in_=o_sb[:])

    nc.compile()
    return nc


def _get_program():
    if "nc" not in _CACHE:
        _CACHE["nc"] = _build_program()
    return _CACHE["nc"]


def _hadamard128():
    idx = np.arange(128, dtype=np.int64)
    anded = idx[:, None] & idx[None, :]
    pc = np.zeros_like(anded)
    for b in range(7):
        pc += (anded >> b) & 1
    return (1.0 - 2.0 * (pc % 2)).astype(np.float32)


def _make_in_maps(hidden_states, w_gate, w_up, w_down):
    x2 = np.ascontiguousarray(hidden_states.reshape(B * S, H),
                              dtype=np.float32)
    wgT = np.ascontiguousarray(np.asarray(w_gate, dtype=np.float32).T)
    wuT = np.ascontiguousarray(np.asarray(w_up, dtype=np.float32).T)
    wdT = np.ascontiguousarray(np.asarray(w_down, dtype=np.float32).T)
    h128 = _hadamard128()

    ci = I // 8
    ch = H // 8
    in_maps = [
        {
            "x": np.ascontiguousarray(x2[T * j:T * (j + 1)]),
            "wgT": wgT, "wuT": wuT, "wdT": wdT, "h128": h128,
            "wg_pre": np.ascontiguousarray(wgT[:, ci * j:ci * (j + 1)]),
            "wu_pre": np.ascontiguousarray(wuT[:, ci * j:ci * (j + 1)]),
            "wd_pre": np.ascontiguousarray(wdT[:, ch * j:ch * (j + 1)]),
        }
        for j in range(N_CORES)
    ]
    return in_maps


def kernel(hidden_states, w_gate, w_up, w_down, _trace=False):
    from concourse.bass_utils import run_bass_kernel_spmd

    nc = _get_program()
    in_maps = _make_in_maps(hidden_states, w_gate, w_up, w_down)
    res = run_bass_kernel_spmd(nc, in_maps, list(range(N_CORES)),
                               trace=_trace)
    pieces = [res.results[j]["out"] for j in range(N_CORES)]
    out = np.concatenate(pieces, axis=0).reshape(B, S, H)
    out = np.ascontiguousarray(out, dtype=np.float32)
    if _trace:
        return out, res
    return out
